# revision 21
# baseline (speedup 1.0000x reference)
"""CrossAttention Trainium2 kernel, 8-core SPMD, bf16 single-pass matmuls.

Sharding: core c -> (batch b = c//2, head-group g = c%2).  Each core computes
8 of the 16 heads for one batch: q/k/v projections restricted to its
inner-dim slice [g*512:(g+1)*512], full attention for those heads, and a
partial output projection (contraction over its 512 inner dims).  Host casts
inputs to bf16 and sums the two partial fp32 outputs per batch + bias.

Precision: tolerance is 2e-2 rel; bf16 matmuls (fp32 PSUM accumulation) land
~3e-3, so every matmul is a single bf16 pass (1 PE cycle/row) instead of the
3-pass fp32r hi/lo split - 3x less PE work, half the ACT exp work, and no
hi/lo vector traffic.  (fp8 DoubleRow measured no faster than bf16 on this
compile path and breached the error gate - see kernel_fp8_attempt.py.bak.)

Schedule: everything is SBUF-resident between the input loads and the output
stores.  K^T and V project first (V with a ones column at h*65+64 so the
softmax denominator rides the attn@v matmul).  The per-n-chunk work is
software-pipelined for the PE: sims run one head ahead of attn@v, and
qproj(jn+1) / oproj(jn-1) chunks are spliced between heads as always-ready
PE filler, so the PE never stalls on the ACT exp backlog (PE ~88% busy).
qT/aoT live in per-jn rotating tiles to avoid false WAR serialization.
Per head: simT[m,n] = kct_h^T q_h (K=64) -> exp on ACT -> bf16 es ->
po[65,512] = va_h^T es -> reciprocal_approx_fast(denominator row; needs an
SBUF staging copy, it misreads PSUM) -> gpsimd partition_broadcast ->
aoT = po * pbs (bf16).  Tail oproj chunks copy PSUM->SBUF on the scalar
engine (idle then) so the psum pool rotation never waits on the DVE queue.
"""
import sys

sys.path.insert(0, "/opt/trn_rl_repo")

import numpy as np
import ml_dtypes

import concourse.bacc as bacc
import concourse.mybir as mybir
import concourse.tile as tile
from concourse.bass_utils import run_bass_kernel_spmd

# bass_utils imports antenv.axon_hooks when trace=True; register a no-op stub
# if the antenv package in this image lacks it.
try:
    import antenv.axon_hooks  # noqa: F401
except ImportError:
    import types as _types

    _stub = _types.ModuleType("antenv.axon_hooks")
    _stub.get_axon_ntff_profile_hook = lambda: None
    _stub.set_axon_ntff_profile_hook = lambda h: None
    sys.modules["antenv.axon_hooks"] = _stub

F32 = mybir.dt.float32
BF16 = mybir.dt.bfloat16
EXP = mybir.ActivationFunctionType.Exp
CPY = mybir.ActivationFunctionType.Copy

B, N, M = 4, 2048, 1024
QD, CD = 1024, 768
HEADS, DH = 16, 64
INNER = HEADS * DH
HG = 8            # heads per core
IS = HG * DH      # inner slice per core = 512
NC = 8

KQ = QD // 128    # 8
KC = CD // 128    # 6
NJ = N // 512     # 4
MT = M // 128     # 8
IT = IS // 128    # 4

LAST_RESULTS = None  # stashed BassKernelResults for test.py introspection


def build_nc():
    nc = bacc.Bacc("TRN2", target_bir_lowering=False, debug=False, num_devices=NC)

    def din(name, shape):
        return nc.dram_tensor(name, shape, BF16, kind="ExternalInput").ap()

    xT = din("xT", [QD, N])
    cT = din("cT", [CD, M])
    wq = din("wq", [QD, IS])
    wk = din("wk", [CD, IS])
    wv = din("wv", [CD, IS])
    wo = din("wo", [IS, QD])
    out = nc.dram_tensor("out", [N, QD], F32, kind="ExternalOutput").ap()

    with tile.TileContext(nc) as tc:
        with tc.tile_pool(name="wp", bufs=1) as wp, \
             tc.tile_pool(name="xp", bufs=1) as xp, \
             tc.tile_pool(name="kv", bufs=1) as kvp, \
             tc.tile_pool(name="qp", bufs=2) as qp, \
             tc.tile_pool(name="ao", bufs=2) as aop, \
             tc.tile_pool(name="es", bufs=2) as esp, \
             tc.tile_pool(name="sm", bufs=3) as smp, \
             tc.tile_pool(name="os", bufs=2) as osp, \
             tc.tile_pool(name="ps", bufs=2, space="PSUM") as psp, \
             tc.tile_pool(name="pq", bufs=2, space="PSUM") as pqp, \
             tc.tile_pool(name="po", bufs=2, space="PSUM") as pop:

            # ---------------- resident input loads ----------------
            wq_sb = [wp.tile([128, IS], BF16, tag=f"wq{k}", name=f"wq{k}")
                     for k in range(KQ)]
            wk_sb = [wp.tile([128, IS], BF16, tag=f"wk{k}", name=f"wk{k}")
                     for k in range(KC)]
            wv_sb = [wp.tile([128, IS], BF16, tag=f"wv{k}", name=f"wv{k}")
                     for k in range(KC)]
            wo_sb = [wp.tile([128, QD], BF16, tag=f"wo{k}", name=f"wo{k}")
                     for k in range(IT)]
            xT_sb = [xp.tile([128, N], BF16, tag=f"xT{k}", name=f"xT{k}")
                     for k in range(KQ)]
            cT_sb = [xp.tile([128, M], BF16, tag=f"cT{k}", name=f"cT{k}")
                     for k in range(KC)]
            for k in range(KC):
                ksl = slice(k * 128, (k + 1) * 128)
                nc.sync.dma_start(wk_sb[k][:], wk[ksl, :])
                nc.sync.dma_start(cT_sb[k][:], cT[ksl, :])
            for k in range(KC):
                ksl = slice(k * 128, (k + 1) * 128)
                nc.sync.dma_start(wv_sb[k][:], wv[ksl, :])
            for k in range(KQ):
                ksl = slice(k * 128, (k + 1) * 128)
                nc.sync.dma_start(wq_sb[k][:], wq[ksl, :])
            for jn in range(NJ):
                nsl = slice(jn * 512, (jn + 1) * 512)
                for k in range(KQ):
                    ksl = slice(k * 128, (k + 1) * 128)
                    nc.sync.dma_start(xT_sb[k][:, nsl], xT[ksl, nsl])
            for k in range(IT):
                nc.sync.dma_start(wo_sb[k][:], wo[k * 128:(k + 1) * 128, :])

            kct_sb = [kvp.tile([128, M], BF16, tag=f"kct{t}", name=f"kct{t}")
                      for t in range(IT)]
            va_sb = [kvp.tile([128, HG * 65], BF16, tag=f"va{mi}",
                              name=f"va{mi}") for mi in range(MT)]


            # ---------------- K^T -> kct ----------------
            with nc.named_scope("kproj"):
                for jm in range(M // 512):
                    msl = slice(jm * 512, (jm + 1) * 512)
                    for mi in range(IT):
                        isl = slice(mi * 128, (mi + 1) * 128)
                        ps = pqp.tile([128, 512], F32, tag="pq", name="pq")
                        for k in range(KC):
                            nc.tensor.matmul(ps[:], wk_sb[k][:, isl],
                                             cT_sb[k][:, msl],
                                             start=(k == 0), stop=(k == KC - 1))
                        nc.vector.tensor_copy(kct_sb[mi][:, msl], ps[:])

            # ---------------- V -> va (ones col rides along) ----------------
            with nc.named_scope("vproj"):
                for mi in range(MT):
                    nc.vector.memset(va_sb[mi][:], 1.0)
                for mi in range(MT):
                    csl = slice(mi * 128, (mi + 1) * 128)
                    ps = pqp.tile([128, 512], F32, tag="pq", name="pq")
                    for k in range(KC):
                        nc.tensor.matmul(ps[:], cT_sb[k][:, csl], wv_sb[k][:],
                                         start=(k == 0), stop=(k == KC - 1))
                    vcol = va_sb[mi][:].rearrange("p (h c) -> p h c", c=65)
                    psv = ps[:].rearrange("p (h c) -> p h c", c=64)
                    nc.vector.tensor_copy(vcol[:, :, 0:64], psv[:])

            # ---- per n-chunk, software-pipelined ----
            # sims run one head ahead of attn@v; qproj(jn+1) and oproj(jn-1)
            # chunks are spliced between heads as always-ready PE filler so
            # the PE never stalls on the ACT exp backlog.
            qts = {}   # jn -> [qT tiles]
            aos = {}   # jn -> [aoT tiles]

            def qproj_chunk(jn, mi):
                nsl = slice(jn * 512, (jn + 1) * 512)
                isl = slice(mi * 128, (mi + 1) * 128)
                with nc.named_scope("qproj"):
                    ps = pqp.tile([128, 512], F32, tag="pq", name="pq")
                    for k in range(KQ):
                        nc.tensor.matmul(ps[:], wq_sb[k][:, isl],
                                         xT_sb[k][:, nsl],
                                         start=(k == 0), stop=(k == KQ - 1))
                    nc.vector.tensor_copy(qts[jn][mi][:], ps[:])

            def oproj_chunk(jn, nt, act_copy=False):
                tsl = slice(nt * 128, (nt + 1) * 128)
                osl = slice(jn * 512 + nt * 128, jn * 512 + nt * 128 + 128)
                with nc.named_scope("oproj"):
                    ob = osp.tile([128, QD], F32, tag="ob", name="ob")
                    for half in range(QD // 512):
                        qsl = slice(half * 512, (half + 1) * 512)
                        ps = pqp.tile([128, 512], F32, tag="pq", name="pq")
                        for k in range(IT):
                            nc.tensor.matmul(ps[:], aos[jn][k][:, tsl],
                                             wo_sb[k][:, qsl],
                                             start=(k == 0), stop=(k == IT - 1))
                        if act_copy:
                            # tail chunks: ACT is idle there, and DVE-copy
                            # latency would serialize the pq psum rotation
                            nc.scalar.activation(ob[:, qsl], ps[:], CPY)
                        else:
                            nc.vector.tensor_copy(ob[:, qsl], ps[:])
                        nc.sync.dma_start(out[osl, qsl], ob[:, qsl])

            def sim_exp(jn, h):
                hp, ro = h // 2, (h % 2) * 64
                rsl = slice(ro, ro + 64)
                es_t = []
                for half in range(MT // 2):
                    ps = psp.tile([128, 1024], F32, tag="ps2", name="ps2")
                    for sub in range(2):
                        mi = 2 * half + sub
                        msl = slice(mi * 128, (mi + 1) * 128)
                        nc.tensor.matmul(ps[:, sub * 512:(sub + 1) * 512],
                                         kct_sb[hp][rsl, msl],
                                         qts[jn][hp][rsl, :],
                                         start=True, stop=True)
                    es = esp.tile([128, 1024], BF16, tag=f"es{half}",
                                  name=f"es{half}", bufs=5)
                    nc.scalar.activation(es[:], ps[:], EXP)
                    es_t.append(es)
                return es_t

            def attn_tail(jn, h, es_t):
                hp, ro = h // 2, (h % 2) * 64
                rsl = slice(ro, ro + 64)
                po = pop.tile([65, 512], F32, tag="po", name="po")
                for mi in range(MT):
                    nc.tensor.matmul(po[:],
                                     va_sb[mi][:, h * 65:h * 65 + 65],
                                     es_t[mi // 2][:, (mi % 2) * 512:
                                                   (mi % 2) * 512 + 512],
                                     start=(mi == 0), stop=(mi == MT - 1))
                # reciprocal_approx_fast misreads PSUM inputs; stage the
                # denominator row through SBUF first.
                dn = smp.tile([1, 512], F32, tag="dn", name="dn")
                nc.vector.tensor_copy(dn[:], po[64:65, :])
                rf = smp.tile([1, 512], F32, tag="rf", name="rf")
                nc.vector.reciprocal_approx_fast(out=rf[:], in_=dn[:])
                pbs = smp.tile([64, 512], F32, tag="pbs", name="pbs")
                nc.gpsimd.partition_broadcast(pbs[:], rf[:])
                nc.vector.tensor_mul(aos[jn][hp][rsl, :], po[0:64, :], pbs[:])

            qts[0] = [qp.tile([128, 512], BF16, tag=f"qT{t}", name=f"qT{t}")
                      for t in range(IT)]
            for mi in range(IT):
                qproj_chunk(0, mi)
            for jn in range(NJ):
                aos[jn] = [aop.tile([128, 512], BF16, tag=f"aoT{t}",
                                    name=f"aoT{t}") for t in range(IT)]
                fillers = []
                if jn + 1 < NJ:
                    qts[jn + 1] = [qp.tile([128, 512], BF16, tag=f"qT{t}",
                                           name=f"qT{t}") for t in range(IT)]
                    fillers += [(qproj_chunk, jn + 1, mi) for mi in range(IT)]
                if jn > 0:
                    fillers += [(oproj_chunk, jn - 1, nt) for nt in range(4)]
                nfill = len(fillers) - (2 if jn == NJ - 1 else 0)
                with nc.named_scope("attn"):
                    es_cur = sim_exp(jn, 0)
                    for h in range(HG):
                        es_nxt = sim_exp(jn, h + 1) if h + 1 < HG else None
                        attn_tail(jn, h, es_cur)
                        es_cur = es_nxt
                        if fillers and h >= HG - nfill:
                            fn, a1, a2 = fillers.pop(0)
                            fn(a1, a2)
                for fn, a1, a2 in fillers:
                    fn(a1, a2, act_copy=True)
            for nt in range(4):
                oproj_chunk(NJ - 1, nt, act_copy=True)
    nc.compile()
    return nc


_NC_CACHE = None


def kernel(x, context, Wq, Wk, Wv, Wo, bo, _trace=False):
    global _NC_CACHE, LAST_RESULTS
    x = np.asarray(x, np.float32)
    context = np.asarray(context, np.float32)
    scale = np.float32(DH ** -0.5)

    if _NC_CACHE is None:
        _NC_CACHE = build_nc()
    nc = _NC_CACHE

    bf = lambda a: np.ascontiguousarray(a).astype(ml_dtypes.bfloat16)

    in_maps = []
    for c in range(NC):
        b, g = c // 2, c % 2
        sl = slice(g * IS, (g + 1) * IS)
        m = {
            "xT": bf(x[b].T),
            "cT": bf(context[b].T),
            "wq": bf(np.asarray(Wq, np.float32)[:, sl] * scale),
            "wk": bf(np.asarray(Wk, np.float32)[:, sl]),
            "wv": bf(np.asarray(Wv, np.float32)[:, sl]),
            "wo": bf(np.asarray(Wo, np.float32)[sl, :]),
        }
        in_maps.append(m)
    res = run_bass_kernel_spmd(nc, in_maps, core_ids=list(range(NC)),
                               trace=_trace)
    LAST_RESULTS = res
    out = np.empty((B, N, QD), np.float32)
    bo32 = np.asarray(bo, np.float32)
    for b in range(B):
        out[b] = res.results[2 * b]["out"] + res.results[2 * b + 1]["out"] + bo32
    return out


# revision 22
# speedup vs baseline: 1.0128x; 1.0128x over previous
"""CrossAttention Trainium2 kernel, 8-core SPMD, bf16 single-pass matmuls.

Sharding: core c -> (batch b = c//2, head-group g = c%2).  Each core computes
8 of the 16 heads for one batch: q/k/v projections restricted to its
inner-dim slice [g*512:(g+1)*512], full attention for those heads, and a
partial output projection (contraction over its 512 inner dims).  Host casts
inputs to bf16 and sums the two partial fp32 outputs per batch + bias.

Precision: tolerance is 2e-2 rel; bf16 matmuls (fp32 PSUM accumulation) land
~3e-3, so every matmul is a single bf16 pass (1 PE cycle/row) instead of the
3-pass fp32r hi/lo split - 3x less PE work, half the ACT exp work, and no
hi/lo vector traffic.  (fp8 DoubleRow measured no faster than bf16 on this
compile path and breached the error gate - see kernel_fp8_attempt.py.bak.)

Schedule: everything is SBUF-resident between the input loads and the output
stores.  K^T and V project first (V with a ones column at h*65+64 so the
softmax denominator rides the attn@v matmul).  The per-n-chunk work is
software-pipelined for the PE: sims run one head ahead of attn@v, and
qproj(jn+1) / oproj(jn-1) chunks are spliced between heads as always-ready
PE filler, so the PE never stalls on the ACT exp backlog (PE ~88% busy).
qT/aoT live in per-jn rotating tiles to avoid false WAR serialization.
Per head: simT[m,n] = kct_h^T q_h (K=64) -> exp on ACT -> bf16 es ->
po[65,512] = va_h^T es -> reciprocal_approx_fast(denominator row; needs an
SBUF staging copy, it misreads PSUM) -> gpsimd partition_broadcast ->
aoT = po * pbs (bf16).  Tail oproj chunks copy PSUM->SBUF on the scalar
engine (idle then) so the psum pool rotation never waits on the DVE queue.
"""
import sys

sys.path.insert(0, "/opt/trn_rl_repo")

import numpy as np
import ml_dtypes

import concourse.bacc as bacc
import concourse.mybir as mybir
import concourse.tile as tile
from concourse.bass_utils import run_bass_kernel_spmd

# bass_utils imports antenv.axon_hooks when trace=True; register a no-op stub
# if the antenv package in this image lacks it.
try:
    import antenv.axon_hooks  # noqa: F401
except ImportError:
    import types as _types

    _stub = _types.ModuleType("antenv.axon_hooks")
    _stub.get_axon_ntff_profile_hook = lambda: None
    _stub.set_axon_ntff_profile_hook = lambda h: None
    sys.modules["antenv.axon_hooks"] = _stub

F32 = mybir.dt.float32
BF16 = mybir.dt.bfloat16
EXP = mybir.ActivationFunctionType.Exp
CPY = mybir.ActivationFunctionType.Copy

B, N, M = 4, 2048, 1024
QD, CD = 1024, 768
HEADS, DH = 16, 64
INNER = HEADS * DH
HG = 8            # heads per core
IS = HG * DH      # inner slice per core = 512
NC = 8

KQ = QD // 128    # 8
KC = CD // 128    # 6
NJ = N // 512     # 4
MT = M // 128     # 8
IT = IS // 128    # 4

LAST_RESULTS = None  # stashed BassKernelResults for test.py introspection


def build_nc():
    nc = bacc.Bacc("TRN2", target_bir_lowering=False, debug=False, num_devices=NC)

    def din(name, shape):
        return nc.dram_tensor(name, shape, BF16, kind="ExternalInput").ap()

    xT = din("xT", [QD, N])
    cT = din("cT", [CD, M])
    wq = din("wq", [QD, IS])
    wk = din("wk", [CD, IS])
    wv = din("wv", [CD, IS])
    wo = din("wo", [IS, QD])
    out = nc.dram_tensor("out", [N, QD], F32, kind="ExternalOutput").ap()

    with tile.TileContext(nc) as tc:
        with tc.tile_pool(name="wp", bufs=1) as wp, \
             tc.tile_pool(name="xp", bufs=1) as xp, \
             tc.tile_pool(name="kv", bufs=1) as kvp, \
             tc.tile_pool(name="qp", bufs=2) as qp, \
             tc.tile_pool(name="ao", bufs=2) as aop, \
             tc.tile_pool(name="es", bufs=2) as esp, \
             tc.tile_pool(name="sm", bufs=2) as smp, \
             tc.tile_pool(name="os", bufs=2) as osp, \
             tc.tile_pool(name="ps", bufs=2, space="PSUM") as psp, \
             tc.tile_pool(name="pq", bufs=2, space="PSUM") as pqp, \
             tc.tile_pool(name="po", bufs=2, space="PSUM") as pop:

            # ---------------- resident input loads ----------------
            wq_sb = [wp.tile([128, IS], BF16, tag=f"wq{k}", name=f"wq{k}")
                     for k in range(KQ)]
            wk_sb = [wp.tile([128, IS], BF16, tag=f"wk{k}", name=f"wk{k}")
                     for k in range(KC)]
            wv_sb = [wp.tile([128, IS], BF16, tag=f"wv{k}", name=f"wv{k}")
                     for k in range(KC)]
            wo_sb = [wp.tile([128, QD], BF16, tag=f"wo{k}", name=f"wo{k}")
                     for k in range(IT)]
            xT_sb = [xp.tile([128, N], BF16, tag=f"xT{k}", name=f"xT{k}")
                     for k in range(KQ)]
            cT_sb = [xp.tile([128, M], BF16, tag=f"cT{k}", name=f"cT{k}")
                     for k in range(KC)]
            for k in range(KC):
                ksl = slice(k * 128, (k + 1) * 128)
                nc.sync.dma_start(wk_sb[k][:], wk[ksl, :])
                nc.sync.dma_start(cT_sb[k][:], cT[ksl, :])
            for k in range(KC):
                ksl = slice(k * 128, (k + 1) * 128)
                nc.sync.dma_start(wv_sb[k][:], wv[ksl, :])
            for k in range(KQ):
                ksl = slice(k * 128, (k + 1) * 128)
                nc.sync.dma_start(wq_sb[k][:], wq[ksl, :])
            for jn in range(NJ):
                nsl = slice(jn * 512, (jn + 1) * 512)
                for k in range(KQ):
                    ksl = slice(k * 128, (k + 1) * 128)
                    nc.sync.dma_start(xT_sb[k][:, nsl], xT[ksl, nsl])
            for k in range(IT):
                nc.sync.dma_start(wo_sb[k][:], wo[k * 128:(k + 1) * 128, :])

            kct_sb = [kvp.tile([128, M], BF16, tag=f"kct{t}", name=f"kct{t}")
                      for t in range(IT)]
            va_sb = [kvp.tile([128, HG * 65], BF16, tag=f"va{mi}",
                              name=f"va{mi}") for mi in range(MT)]


            # ---------------- K^T -> kct ----------------
            with nc.named_scope("kproj"):
                for jm in range(M // 512):
                    msl = slice(jm * 512, (jm + 1) * 512)
                    for mi in range(IT):
                        isl = slice(mi * 128, (mi + 1) * 128)
                        ps = pqp.tile([128, 512], F32, tag="pq", name="pq")
                        for k in range(KC):
                            nc.tensor.matmul(ps[:], wk_sb[k][:, isl],
                                             cT_sb[k][:, msl],
                                             start=(k == 0), stop=(k == KC - 1))
                        nc.vector.tensor_copy(kct_sb[mi][:, msl], ps[:])

            # ---------------- V -> va (ones col rides along) ----------------
            with nc.named_scope("vproj"):
                for mi in range(MT):
                    nc.vector.memset(va_sb[mi][:], 1.0)
                for mi in range(MT):
                    csl = slice(mi * 128, (mi + 1) * 128)
                    ps = pqp.tile([128, 512], F32, tag="pq", name="pq")
                    for k in range(KC):
                        nc.tensor.matmul(ps[:], cT_sb[k][:, csl], wv_sb[k][:],
                                         start=(k == 0), stop=(k == KC - 1))
                    vcol = va_sb[mi][:].rearrange("p (h c) -> p h c", c=65)
                    psv = ps[:].rearrange("p (h c) -> p h c", c=64)
                    nc.vector.tensor_copy(vcol[:, :, 0:64], psv[:])

            # ---- per n-chunk, software-pipelined ----
            # sims run one head ahead of attn@v; qproj(jn+1) and oproj(jn-1)
            # chunks are spliced between heads as always-ready PE filler so
            # the PE never stalls on the ACT exp backlog.
            qts = {}   # jn -> [qT tiles]
            aos = {}   # jn -> [aoT tiles]

            def qproj_chunk(jn, mi):
                nsl = slice(jn * 512, (jn + 1) * 512)
                isl = slice(mi * 128, (mi + 1) * 128)
                with nc.named_scope("qproj"):
                    ps = pqp.tile([128, 512], F32, tag="pq", name="pq")
                    for k in range(KQ):
                        nc.tensor.matmul(ps[:], wq_sb[k][:, isl],
                                         xT_sb[k][:, nsl],
                                         start=(k == 0), stop=(k == KQ - 1))
                    nc.vector.tensor_copy(qts[jn][mi][:], ps[:])

            def oproj_chunk(jn, nt, act_copy=False):
                tsl = slice(nt * 128, (nt + 1) * 128)
                osl = slice(jn * 512 + nt * 128, jn * 512 + nt * 128 + 128)
                with nc.named_scope("oproj"):
                    ob = osp.tile([128, QD], F32, tag="ob", name="ob")
                    for half in range(QD // 512):
                        qsl = slice(half * 512, (half + 1) * 512)
                        ps = pqp.tile([128, 512], F32, tag="pq", name="pq")
                        for k in range(IT):
                            nc.tensor.matmul(ps[:], aos[jn][k][:, tsl],
                                             wo_sb[k][:, qsl],
                                             start=(k == 0), stop=(k == IT - 1))
                        if act_copy:
                            # tail chunks: ACT is idle there, and DVE-copy
                            # latency would serialize the pq psum rotation
                            nc.scalar.activation(ob[:, qsl], ps[:], CPY)
                        else:
                            nc.vector.tensor_copy(ob[:, qsl], ps[:])
                        nc.sync.dma_start(out[osl, qsl], ob[:, qsl])

            def sim_exp(jn, h):
                hp, ro = h // 2, (h % 2) * 64
                rsl = slice(ro, ro + 64)
                es_t = []
                for half in range(MT // 2):
                    ps = psp.tile([128, 1024], F32, tag="ps2", name="ps2")
                    for sub in range(2):
                        mi = 2 * half + sub
                        msl = slice(mi * 128, (mi + 1) * 128)
                        nc.tensor.matmul(ps[:, sub * 512:(sub + 1) * 512],
                                         kct_sb[hp][rsl, msl],
                                         qts[jn][hp][rsl, :],
                                         start=True, stop=True)
                    es = esp.tile([128, 1024], BF16, tag=f"es{half}",
                                  name=f"es{half}", bufs=4)
                    nc.scalar.activation(es[:], ps[:], EXP)
                    es_t.append(es)
                return es_t

            def attn_tail(jn, h, es_t):
                hp, ro = h // 2, (h % 2) * 64
                rsl = slice(ro, ro + 64)
                po = pop.tile([65, 512], F32, tag="po", name="po")
                for mi in range(MT):
                    nc.tensor.matmul(po[:],
                                     va_sb[mi][:, h * 65:h * 65 + 65],
                                     es_t[mi // 2][:, (mi % 2) * 512:
                                                   (mi % 2) * 512 + 512],
                                     start=(mi == 0), stop=(mi == MT - 1))
                # reciprocal_approx_fast misreads PSUM inputs; stage the
                # denominator row through SBUF first.
                dn = smp.tile([1, 512], F32, tag="dn", name="dn")
                nc.vector.tensor_copy(dn[:], po[64:65, :])
                rf = smp.tile([1, 512], F32, tag="rf", name="rf")
                nc.vector.reciprocal_approx_fast(out=rf[:], in_=dn[:])
                pbs = smp.tile([64, 512], F32, tag="pbs", name="pbs")
                nc.gpsimd.partition_broadcast(pbs[:], rf[:])
                nc.vector.tensor_mul(aos[jn][hp][rsl, :], po[0:64, :], pbs[:])

            qts[0] = [qp.tile([128, 512], BF16, tag=f"qT{t}", name=f"qT{t}")
                      for t in range(IT)]
            for mi in range(IT):
                qproj_chunk(0, mi)
            for jn in range(NJ):
                aos[jn] = [aop.tile([128, 512], BF16, tag=f"aoT{t}",
                                    name=f"aoT{t}") for t in range(IT)]
                fillers = []
                if jn + 1 < NJ:
                    qts[jn + 1] = [qp.tile([128, 512], BF16, tag=f"qT{t}",
                                           name=f"qT{t}") for t in range(IT)]
                    fillers += [(qproj_chunk, jn + 1, mi) for mi in range(IT)]
                if jn > 0:
                    fillers += [(oproj_chunk, jn - 1, nt) for nt in range(4)]
                nfill = len(fillers) - (2 if jn == NJ - 1 else 0)
                with nc.named_scope("attn"):
                    es_cur = sim_exp(jn, 0)
                    for h in range(HG):
                        es_nxt = sim_exp(jn, h + 1) if h + 1 < HG else None
                        attn_tail(jn, h, es_cur)
                        es_cur = es_nxt
                        if fillers and h >= HG - nfill:
                            fn, a1, a2 = fillers.pop(0)
                            fn(a1, a2)
                for fn, a1, a2 in fillers:
                    fn(a1, a2, act_copy=True)
            for nt in range(4):
                oproj_chunk(NJ - 1, nt, act_copy=True)
    nc.compile()
    return nc


_NC_CACHE = None


def kernel(x, context, Wq, Wk, Wv, Wo, bo, _trace=False):
    global _NC_CACHE, LAST_RESULTS
    x = np.asarray(x, np.float32)
    context = np.asarray(context, np.float32)
    scale = np.float32(DH ** -0.5)

    if _NC_CACHE is None:
        _NC_CACHE = build_nc()
    nc = _NC_CACHE

    bf = lambda a: np.ascontiguousarray(a).astype(ml_dtypes.bfloat16)

    in_maps = []
    for c in range(NC):
        b, g = c // 2, c % 2
        sl = slice(g * IS, (g + 1) * IS)
        m = {
            "xT": bf(x[b].T),
            "cT": bf(context[b].T),
            "wq": bf(np.asarray(Wq, np.float32)[:, sl] * scale),
            "wk": bf(np.asarray(Wk, np.float32)[:, sl]),
            "wv": bf(np.asarray(Wv, np.float32)[:, sl]),
            "wo": bf(np.asarray(Wo, np.float32)[sl, :]),
        }
        in_maps.append(m)
    res = run_bass_kernel_spmd(nc, in_maps, core_ids=list(range(NC)),
                               trace=_trace)
    LAST_RESULTS = res
    out = np.empty((B, N, QD), np.float32)
    bo32 = np.asarray(bo, np.float32)
    for b in range(B):
        out[b] = res.results[2 * b]["out"] + res.results[2 * b + 1]["out"] + bo32
    return out
